# revision 1
# baseline (speedup 1.0000x reference)
"""EnhancedLSTMCell Trainium2 kernel.

Data-parallel over 8 NeuronCores: batch B=8192 split into 8 shards of 1024
rows. Per core:
    gates = [x | h_prev] @ W + b          # [1024, 4096] via PE, fp32r
    i,f,g,o = split(gates); f *= mask
    c = f*c_prev + i*g; c = LayerNorm(c)*gamma + beta; h = o*tanh(c)

Layout: batch rows on partitions (8 chunks of 128), contraction dim K=2048 on
partitions for matmul operands. The host feeds [x | h_prev]^T per shard so
the contraction dim lands on partitions with unit-stride DMA; tiles are
rounded to fp32r (full-rate PE at free-dim >= 256) by DVE copies. W is
streamed once in 16 column-slices of 256, each as four pipelined quarter-K
chunks (fp32 staging -> fp32r via ACT) with a one-block prefetch queue.
Bias enters each PSUM accumulation via a leading K=1 ones-row matmul; ACT
consumes gate pre-activations straight from PSUM. c accumulates in SBUF: the i-drain
writes sigmoid(i) in place, the g-drain multiplies tanh(g) in, the f-drain
adds (sigmoid(f)*mask)*c_prev. LayerNorm uses bn_stats/bn_aggr + Sqrt +
DVE reciprocal; tanh(c_t) overwrites the accumulator to feed
h = sigmoid(o) * tanh(c_t).

Built on bacc.Bacc (not bass.Bass): Bacc's finalize() legalizes multi-sem
waits that the walrus DMA/LDW instruction encodings cannot carry.
"""

import sys

if "/opt/trn_rl_repo" not in sys.path:
    sys.path.insert(0, "/opt/trn_rl_repo")

import numpy as np

B = 8192
IN = 1024
H = 1024
NCORES = 8
BC = B // NCORES          # 1024 rows per core
MCH = BC // 128           # 8 partition chunks of batch rows
KCH = (IN + H) // 128     # 16 contraction chunks
CB = 256                  # W column-block width
EPS = 1e-5

_PROGRAMS = {}


def _build_program(trivial_gb: bool):
    from contextlib import ExitStack

    import concourse.bass as bass
    import concourse.tile as tile
    from concourse import bacc, mybir

    F32 = mybir.dt.float32
    F32R = mybir.dt.float32r
    AF = mybir.ActivationFunctionType
    ALU = mybir.AluOpType

    nc = bacc.Bacc("TRN2", target_bir_lowering=False, debug=False)

    # combined^T = [x | h_prev]^T per shard, transposed host-side during
    # sharding so the contraction dim lands on partitions with unit-stride DMA
    ct_d = nc.dram_tensor("combT", [IN + H, BC], F32, kind="ExternalInput").ap()
    c_d = nc.dram_tensor("c_prev", [BC, H], F32, kind="ExternalInput").ap()
    m_d = nc.dram_tensor("forget_mask", [MCH, 128], F32, kind="ExternalInput").ap()
    w_d = nc.dram_tensor("W", [IN + H, 4 * H], F32, kind="ExternalInput").ap()
    b_d = nc.dram_tensor("b", [1, 4 * H], F32, kind="ExternalInput").ap()
    g_d = nc.dram_tensor("ln_gamma", [1, H], F32, kind="ExternalInput").ap()
    be_d = nc.dram_tensor("ln_beta", [1, H], F32, kind="ExternalInput").ap()
    ho_d = nc.dram_tensor("h_out", [BC, H], F32, kind="ExternalOutput").ap()
    co_d = nc.dram_tensor("c_out", [BC, H], F32, kind="ExternalOutput").ap()

    w_k = w_d.rearrange("(k p) n -> p k n", p=128)  # [128, 16, 4096]
    ct_k = ct_d.rearrange("(k p) b -> p k b", p=128)  # [128, 16, 1024]

    with tile.TileContext(nc) as tc, ExitStack() as ctx:
        singles = ctx.enter_context(tc.tile_pool(name="singles", bufs=1))
        bigs = ctx.enter_context(tc.tile_pool(name="bigs", bufs=1))
        wpool = ctx.enter_context(tc.tile_pool(name="w", bufs=4))
        wrpool = ctx.enter_context(tc.tile_pool(name="wr", bufs=2))
        ctpool = ctx.enter_context(tc.tile_pool(name="ctstage", bufs=2))
        tpool = ctx.enter_context(tc.tile_pool(name="tmp", bufs=4))
        cppool = ctx.enter_context(tc.tile_pool(name="cprev", bufs=3))
        hpool = ctx.enter_context(tc.tile_pool(name="hout", bufs=3))
        zpool = ctx.enter_context(
            tc.tile_pool(name="z", bufs=2 if trivial_gb else 1))
        pmain = ctx.enter_context(tc.tile_pool(name="pmain", bufs=8, space="PSUM"))

        # bias enters PSUM via a K=1 ones-row matmul (start=True), so the
        # k-loop accumulates on top and ACT drains see gates+bias directly.
        # (A DVE post-add was measured slower: it sits in every psum->drain
        # chain, costing more than the 14us of PE the ones-matmuls use.)
        ones_r = singles.tile([1, 128], F32R)
        b_r = singles.tile([1, 4 * H], F32R)
        with tc.tile_pool(name="stage", bufs=1) as stage:
            ones_f = stage.tile([1, 128], F32)
            nc.vector.memset(ones_f, 1.0)
            nc.scalar.copy(ones_r, ones_f)
            b_stage = stage.tile([1, 4 * H], F32)
            nc.sync.dma_start(out=b_stage, in_=b_d)
            nc.scalar.copy(b_r, b_stage)
        mask_sb = singles.tile([128, MCH], F32)
        nc.sync.dma_start(out=mask_sb, in_=m_d.rearrange("m p -> p m"))
        if not trivial_gb:
            gam_bc = singles.tile([128, H], F32)
            nc.sync.dma_start(
                out=gam_bc,
                in_=bass.AP(tensor=g_d.tensor, offset=g_d.offset,
                            ap=[[0, 128], g_d.ap[1]]),
            )
            bet_bc = singles.tile([128, H], F32)
            nc.sync.dma_start(
                out=bet_bc,
                in_=bass.AP(tensor=be_d.tensor, offset=be_d.offset,
                            ap=[[0, 128], be_d.ap[1]]),
            )

        # combT[k, m] = (128x128 transposed block of [x | h_prev]), stored
        # pre-rounded to fp32r for the PE
        combT = bigs.tile([128, KCH, MCH, 128], F32R)
        c_acc = bigs.tile([128, MCH, H], F32)
        mvall = singles.tile([128, MCH, 2], F32)
        std_t = singles.tile([128, MCH], F32)
        inv_t = singles.tile([128, MCH], F32)
        eps_t = singles.tile([128, 1], F32)
        nc.vector.memset(eps_t, EPS)

        # ---- main loop over W column blocks ----
        # order: (i,g) interleaved per quarter, then f, then LN, then o
        GOFF = {"i": 0, "f": H, "g": 2 * H, "o": 3 * H}
        NQ = H // CB  # quarters per gate
        blocks = []
        for q in range(NQ):
            blocks.append(("i", q))
            blocks.append(("g", q))
        blocks += [("f", q) for q in range(NQ)]
        o_blocks = [("o", q) for q in range(NQ)]

        def load_w(gate, q):
            # W slice load in four pipelined quarter-K chunks: matmuls for
            # the first k-tiles start as soon as the first chunk is rounded,
            # and chunk DMAs of the next block overlap the current block.
            col0 = GOFF[gate] + q * CB
            wr = wrpool.tile([128, KCH, CB], F32R, tag="wr")
            hk = KCH // 4
            for hchunk in range(4):
                wt = wpool.tile([128, hk, CB], F32, tag="w")
                nc.sync.dma_start(
                    out=wt,
                    in_=w_k[:, hchunk * hk:(hchunk + 1) * hk, col0:col0 + CB])
                # fp32 -> fp32r rounding on ACT
                nc.scalar.copy(wr[:, hchunk * hk:(hchunk + 1) * hk, :], wt)
            return wr

        def do_block(gate, q, wr):
            col0 = GOFF[gate] + q * CB
            for m in range(MCH):
                ps = pmain.tile([128, CB], F32, tag="ps")
                nc.tensor.matmul(ps, ones_r, b_r[:, col0:col0 + CB],
                                 start=True, stop=False)
                for k in range(KCH):
                    nc.tensor.matmul(
                        ps, combT[:, k, m, :], wr[:, k, :],
                        start=False, stop=(k == KCH - 1),
                    )
                csl = c_acc[:, m, q * CB:(q + 1) * CB]
                if gate == "i":
                    nc.scalar.activation(csl, ps, AF.Sigmoid)
                elif gate == "g":
                    tg = tpool.tile([128, CB], F32, tag="t")
                    nc.scalar.activation(tg, ps, AF.Tanh)
                    nc.vector.tensor_mul(csl, csl, tg)
                elif gate == "f":
                    tf = tpool.tile([128, CB], F32, tag="t")
                    nc.scalar.activation(tf, ps, AF.Sigmoid)
                    cp = cppool.tile([128, CB], F32, tag="cp")
                    nc.sync.dma_start(
                        out=cp,
                        in_=c_d[m * 128:(m + 1) * 128, q * CB:(q + 1) * CB])
                    t2 = tpool.tile([128, CB], F32, tag="t")
                    nc.vector.scalar_tensor_tensor(
                        t2, tf, mask_sb[:, m:m + 1], cp, ALU.mult, ALU.mult)
                    nc.vector.tensor_add(csl, csl, t2)
                else:  # o
                    to = tpool.tile([128, CB], F32, tag="t")
                    nc.scalar.activation(to, ps, AF.Sigmoid)
                    hh = hpool.tile([128, CB], F32, tag="h")
                    nc.vector.tensor_mul(hh, to, csl)  # csl holds tanh(c_t)
                    nc.sync.dma_start(
                        out=ho_d[m * 128:(m + 1) * 128, q * CB:(q + 1) * CB],
                        in_=hh)

        def emit_ln():
            # ---- LayerNorm over H per m-chunk ----
            for m in range(MCH):
                st = tpool.tile([128, 2, 6], F32, tag="st")
                for hf in range(2):
                    nc.vector.bn_stats(
                        out=st[:, hf, :],
                        in_=c_acc[:, m, hf * 512:(hf + 1) * 512])
                nc.vector.bn_aggr(out=mvall[:, m, :], in_=st)
            # std = sqrt(var + eps); inv = 1/std; nmi = -mean*inv
            nc.scalar.activation(std_t, mvall[:, :, 1], AF.Sqrt, bias=eps_t)
            nc.vector.reciprocal(inv_t, std_t)
            for m in range(MCH):
                z = zpool.tile([128, H], F32, tag="z")
                nc.vector.tensor_scalar(
                    z, c_acc[:, m, :], mvall[:, m, 0:1], inv_t[:, m:m + 1],
                    ALU.subtract, ALU.mult)
                if not trivial_gb:
                    nc.vector.tensor_mul(z, z, gam_bc)
                    nc.vector.tensor_add(z, z, bet_bc)
                nc.sync.dma_start(out=co_d[m * 128:(m + 1) * 128, :], in_=z)
                nc.scalar.activation(c_acc[:, m, :], z, AF.Tanh)

        # One-block W prefetch: the next block's W chunks (DMA + fp32r
        # rounding) are emitted before the current block's matmuls, and the
        # first o-block's W is already in flight before the LayerNorm work.
        allb = blocks + o_blocks

        # ---- load combined^T and round to fp32r; the m=0 slab goes first,
        # then the first W slice, then the remaining slabs, so the serial
        # DMA startup chain covers exactly what the first matmuls need ----
        hk0 = KCH // 2

        def load_ct(m):
            for hchunk in range(2):
                cts = ctpool.tile([128, hk0, 128], F32, tag="cts")
                nc.sync.dma_start(
                    out=cts,
                    in_=ct_k[:, hchunk * hk0:(hchunk + 1) * hk0,
                             m * 128:(m + 1) * 128])
                nc.vector.tensor_copy(
                    combT[:, hchunk * hk0:(hchunk + 1) * hk0, m, :], cts)

        load_ct(0)
        wr_next = load_w(*allb[0])
        for m in range(1, MCH):
            load_ct(m)

        ln_done = False
        for idx, (gate, q) in enumerate(allb):
            if gate == "o" and not ln_done:
                emit_ln()
                ln_done = True
            wr_cur = wr_next
            if idx + 1 < len(allb):
                wr_next = load_w(*allb[idx + 1])
            do_block(gate, q, wr_cur)

    nc.finalize()
    return nc


def _get_program(trivial_gb: bool):
    if trivial_gb not in _PROGRAMS:
        _PROGRAMS[trivial_gb] = _build_program(trivial_gb)
    return _PROGRAMS[trivial_gb]


def kernel(x, h_prev, c_prev, forget_mask, W, b, ln_gamma, ln_beta):
    from concourse.bass_utils import run_bass_kernel_spmd

    f32 = np.float32
    x = np.ascontiguousarray(x, dtype=f32)
    h_prev = np.ascontiguousarray(h_prev, dtype=f32)
    c_prev = np.ascontiguousarray(c_prev, dtype=f32)
    forget_mask = np.ascontiguousarray(forget_mask, dtype=f32)
    W = np.ascontiguousarray(W, dtype=f32)
    b = np.ascontiguousarray(b, dtype=f32)
    ln_gamma = np.ascontiguousarray(ln_gamma, dtype=f32)
    ln_beta = np.ascontiguousarray(ln_beta, dtype=f32)

    trivial_gb = bool(np.all(ln_gamma == 1.0) and np.all(ln_beta == 0.0))
    nc = _get_program(trivial_gb)

    # pre-transposed [x | h_prev] per shard: [IN+H, BC], contraction-major
    comb_t = np.ascontiguousarray(
        np.concatenate((x, h_prev), axis=1).T)  # [IN+H, B]

    in_maps = []
    for i in range(NCORES):
        sl = slice(i * BC, (i + 1) * BC)
        in_maps.append({
            "combT": np.ascontiguousarray(comb_t[:, sl]),
            "c_prev": c_prev[sl],
            "forget_mask": forget_mask[sl].reshape(MCH, 128),
            "W": W,
            "b": b.reshape(1, 4 * H),
            "ln_gamma": ln_gamma.reshape(1, H),
            "ln_beta": ln_beta.reshape(1, H),
        })

    res = run_bass_kernel_spmd(nc, in_maps, list(range(NCORES)))
    h_t = np.concatenate([r["h_out"] for r in res.results], axis=0)
    c_t = np.concatenate([r["c_out"] for r in res.results], axis=0)
    return (h_t, c_t)



# revision 2
# speedup vs baseline: 1.2539x; 1.2539x over previous
"""EnhancedLSTMCell Trainium2 kernel.

Data-parallel over 8 NeuronCores: batch B=8192 split into 8 shards of 1024
rows. Per core:
    gates = [x | h_prev] @ W + b          # [1024, 4096]
    i,f,g,o = split(gates); f *= mask
    c = f*c_prev + i*g; c = LayerNorm(c)*gamma + beta; h = o*tanh(c)

The GEMM runs on the PE in fp8e4 (e4m3) DoubleRow perf mode (2 k-tiles per
instruction at 0.5 cycles/row — 4x the fp32r MAC rate). fp32 precision is
recovered with a 2-term split of BOTH operands: v ~ v_hi + v_lo with v_hi =
q8(v) and v_lo = q8(v - v_hi) held at the SAME scale (the residual lands in
e4m3's subnormal range, giving a uniform ~2^-10 absolute floor). Per k-pair
the three significant cross products hi@hi, hi@lo, lo@hi are accumulated
(lo@lo is ~1e-6 relative and dropped), so the GEMM costs 1.5 DR instructions
per k-tile = 0.75x the fp32r cycle count with ~1e-3 relative gate error.
W is pre-scaled by 64 host-side so its fp8 range is normal; the ACT drains
apply scale=1/64. Bias enters each PSUM group via a K=1 DoubleRow matmul of
(ones*4) @ q8(16*b) (= 64*b, rescaled by the same drain). All quantization
happens host-side, so fp8 tiles are DMA'd straight into the PE operands —
no on-device rounding copies at all.

Layout: batch rows on partitions (8 chunks of 128), contraction dim K=2048
on partitions for matmul operands (host feeds [x | h_prev]^T per shard).
c accumulates in SBUF fp32: the i-drain writes sigmoid(i) in place, the
g-drain multiplies tanh(g) in, the f-drain adds (sigmoid(f)*mask)*c_prev
(c_prev DMA'd as bf16). LayerNorm uses bn_stats/bn_aggr + Sqrt + DVE
reciprocal; tanh(c_t) overwrites the accumulator to feed h = sigmoid(o) *
tanh(c_t), written out as bf16 (c_out stays fp32).

Built on bacc.Bacc (not bass.Bass): Bacc's finalize() legalizes multi-sem
waits that the walrus DMA/LDW instruction encodings cannot carry.
"""

import sys

if "/opt/trn_rl_repo" not in sys.path:
    sys.path.insert(0, "/opt/trn_rl_repo")

import numpy as np
import ml_dtypes

B = 8192
IN = 1024
H = 1024
NCORES = 8
BC = B // NCORES          # 1024 rows per core
MCH = BC // 128           # 8 partition chunks of batch rows
KCH = (IN + H) // 128     # 16 contraction chunks
KP = KCH // 2             # 8 DoubleRow k-pairs
CB = 256                  # W column-block width
EPS = 1e-5
WSCALE = 64.0             # W pre-scale; drains divide by 64

E4NP = ml_dtypes.float8_e4m3
BF16 = ml_dtypes.bfloat16

_PROGRAMS = {}


def _build_program(trivial_gb: bool):
    from contextlib import ExitStack

    import concourse.bass as bass
    import concourse.tile as tile
    from concourse import bacc, mybir

    F32 = mybir.dt.float32
    F8 = mybir.dt.float8e4
    BF = mybir.dt.bfloat16
    AF = mybir.ActivationFunctionType
    ALU = mybir.AluOpType
    DR = mybir.MatmulPerfMode.DoubleRow

    nc = bacc.Bacc("TRN2", target_bir_lowering=False, debug=False)

    # combined^T = [x | h_prev]^T per shard, hi/lo fp8 split, transposed
    # host-side so the contraction dim lands on partitions with unit-stride DMA
    cth_d = nc.dram_tensor("combT_hi", [IN + H, BC], F8, kind="ExternalInput").ap()
    ctl_d = nc.dram_tensor("combT_lo", [IN + H, BC], F8, kind="ExternalInput").ap()
    c_d = nc.dram_tensor("c_prev", [BC, H], BF, kind="ExternalInput").ap()
    m_d = nc.dram_tensor("forget_mask", [MCH, 128], F32, kind="ExternalInput").ap()
    wh_d = nc.dram_tensor("W_hi", [IN + H, 4 * H], F8, kind="ExternalInput").ap()
    wl_d = nc.dram_tensor("W_lo", [IN + H, 4 * H], F8, kind="ExternalInput").ap()
    ones_d = nc.dram_tensor("ones_dr", [1, 2, 128], F8, kind="ExternalInput").ap()
    b_d = nc.dram_tensor("b_dr", [1, 2, 4 * H], F8, kind="ExternalInput").ap()
    g_d = nc.dram_tensor("ln_gamma", [1, H], F32, kind="ExternalInput").ap()
    be_d = nc.dram_tensor("ln_beta", [1, H], F32, kind="ExternalInput").ap()
    ho_d = nc.dram_tensor("h_out", [BC, H], BF, kind="ExternalOutput").ap()
    co_d = nc.dram_tensor("c_out", [BC, H], F32, kind="ExternalOutput").ap()

    wh_k = wh_d.rearrange("(k p) n -> p k n", p=128)  # [128, 16, 4096]
    wl_k = wl_d.rearrange("(k p) n -> p k n", p=128)
    cth_k = cth_d.rearrange("(k p) b -> p k b", p=128)  # [128, 16, 1024]
    ctl_k = ctl_d.rearrange("(k p) b -> p k b", p=128)

    with tile.TileContext(nc) as tc, ExitStack() as ctx:
        singles = ctx.enter_context(tc.tile_pool(name="singles", bufs=1))
        bigs = ctx.enter_context(tc.tile_pool(name="bigs", bufs=1))
        wpool = ctx.enter_context(tc.tile_pool(name="w", bufs=4))
        tpool = ctx.enter_context(tc.tile_pool(name="tmp", bufs=4))
        cppool = ctx.enter_context(tc.tile_pool(name="cprev", bufs=3))
        hpool = ctx.enter_context(tc.tile_pool(name="hout", bufs=3))
        zpool = ctx.enter_context(
            tc.tile_pool(name="z", bufs=2 if trivial_gb else 1))
        pmain = ctx.enter_context(tc.tile_pool(name="pmain", bufs=8, space="PSUM"))

        ones_t = singles.tile([1, 2, 128], F8)
        b_t = singles.tile([1, 2, 4 * H], F8)
        nc.sync.dma_start(out=ones_t, in_=ones_d)
        nc.sync.dma_start(out=b_t, in_=b_d)
        mask_sb = singles.tile([128, MCH], F32)
        nc.sync.dma_start(out=mask_sb, in_=m_d.rearrange("m p -> p m"))
        if not trivial_gb:
            gam_bc = singles.tile([128, H], F32)
            nc.sync.dma_start(
                out=gam_bc,
                in_=bass.AP(tensor=g_d.tensor, offset=g_d.offset,
                            ap=[[0, 128], g_d.ap[1]]),
            )
            bet_bc = singles.tile([128, H], F32)
            nc.sync.dma_start(
                out=bet_bc,
                in_=bass.AP(tensor=be_d.tensor, offset=be_d.offset,
                            ap=[[0, 128], be_d.ap[1]]),
            )

        # combT[k, m] = 128x128 transposed blocks of [x | h_prev], fp8 hi/lo
        combH = bigs.tile([128, KCH, MCH, 128], F8)
        combL = bigs.tile([128, KCH, MCH, 128], F8)
        c_acc = bigs.tile([128, MCH, H], F32)
        mvall = singles.tile([128, MCH, 2], F32)
        std_t = singles.tile([128, MCH], F32)
        inv_t = singles.tile([128, MCH], F32)
        eps_t = singles.tile([128, 1], F32)
        nc.vector.memset(eps_t, EPS)

        # ---- main loop over W column blocks ----
        # order: (i,g) interleaved per quarter, then f, then LN, then o
        GOFF = {"i": 0, "f": H, "g": 2 * H, "o": 3 * H}
        NQ = H // CB  # quarters per gate
        blocks = []
        for q in range(NQ):
            blocks.append(("i", q))
            blocks.append(("g", q))
        blocks += [("f", q) for q in range(NQ)]
        o_blocks = [("o", q) for q in range(NQ)]

        def load_w(gate, q):
            # W hi/lo slices for one column block, straight into fp8 PE
            # operands (half-K chunks so the next block's DMAs pipeline).
            col0 = GOFF[gate] + q * CB
            wh = wpool.tile([128, KCH, CB], F8, tag="wh")
            wl = wpool.tile([128, KCH, CB], F8, tag="wl")
            hk = KCH // 2
            for hchunk in range(2):
                ks = slice(hchunk * hk, (hchunk + 1) * hk)
                nc.sync.dma_start(out=wh[:, ks, :],
                                  in_=wh_k[:, ks, col0:col0 + CB])
                nc.sync.dma_start(out=wl[:, ks, :],
                                  in_=wl_k[:, ks, col0:col0 + CB])
            return wh, wl

        def do_block(gate, q, wh, wl):
            col0 = GOFF[gate] + q * CB
            for m in range(MCH):
                ps = pmain.tile([128, CB], F32, tag="ps")
                # bias via K=1 DoubleRow: (ones*4) @ q8(16*b) = 64*b
                nc.tensor.matmul(ps, ones_t, b_t[:, :, col0:col0 + CB],
                                 start=True, stop=False, perf_mode=DR)
                for j in range(KP):
                    ah = combH[:, 2 * j:2 * j + 2, m, :]
                    al = combL[:, 2 * j:2 * j + 2, m, :]
                    whj = wh[:, 2 * j:2 * j + 2, :]
                    wlj = wl[:, 2 * j:2 * j + 2, :]
                    nc.tensor.matmul(ps, ah, whj, start=False, stop=False,
                                     perf_mode=DR)
                    nc.tensor.matmul(ps, ah, wlj, start=False, stop=False,
                                     perf_mode=DR)
                    nc.tensor.matmul(ps, al, whj, start=False,
                                     stop=(j == KP - 1), perf_mode=DR)
                csl = c_acc[:, m, q * CB:(q + 1) * CB]
                inv = 1.0 / WSCALE
                if gate == "i":
                    nc.scalar.activation(csl, ps, AF.Sigmoid, scale=inv)
                elif gate == "g":
                    tg = tpool.tile([128, CB], F32, tag="t")
                    nc.scalar.activation(tg, ps, AF.Tanh, scale=inv)
                    nc.vector.tensor_mul(csl, csl, tg)
                elif gate == "f":
                    tf = tpool.tile([128, CB], F32, tag="t")
                    nc.scalar.activation(tf, ps, AF.Sigmoid, scale=inv)
                    cp = cppool.tile([128, CB], BF, tag="cp")
                    nc.sync.dma_start(
                        out=cp,
                        in_=c_d[m * 128:(m + 1) * 128, q * CB:(q + 1) * CB])
                    t2 = tpool.tile([128, CB], F32, tag="t")
                    nc.vector.scalar_tensor_tensor(
                        t2, tf, mask_sb[:, m:m + 1], cp, ALU.mult, ALU.mult)
                    nc.vector.tensor_add(csl, csl, t2)
                else:  # o
                    to = tpool.tile([128, CB], F32, tag="t")
                    nc.scalar.activation(to, ps, AF.Sigmoid, scale=inv)
                    hh = hpool.tile([128, CB], BF, tag="h")
                    nc.vector.tensor_mul(hh, to, csl)  # csl holds tanh(c_t)
                    nc.sync.dma_start(
                        out=ho_d[m * 128:(m + 1) * 128, q * CB:(q + 1) * CB],
                        in_=hh)

        def emit_ln():
            # ---- LayerNorm over H per m-chunk ----
            for m in range(MCH):
                st = tpool.tile([128, 2, 6], F32, tag="st")
                for hf in range(2):
                    nc.vector.bn_stats(
                        out=st[:, hf, :],
                        in_=c_acc[:, m, hf * 512:(hf + 1) * 512])
                nc.vector.bn_aggr(out=mvall[:, m, :], in_=st)
            # std = sqrt(var + eps); inv = 1/std; nmi = -mean*inv
            nc.scalar.activation(std_t, mvall[:, :, 1], AF.Sqrt, bias=eps_t)
            nc.vector.reciprocal(inv_t, std_t)
            for m in range(MCH):
                z = zpool.tile([128, H], F32, tag="z")
                nc.vector.tensor_scalar(
                    z, c_acc[:, m, :], mvall[:, m, 0:1], inv_t[:, m:m + 1],
                    ALU.subtract, ALU.mult)
                if not trivial_gb:
                    nc.vector.tensor_mul(z, z, gam_bc)
                    nc.vector.tensor_add(z, z, bet_bc)
                nc.sync.dma_start(out=co_d[m * 128:(m + 1) * 128, :], in_=z)
                nc.scalar.activation(c_acc[:, m, :], z, AF.Tanh)

        # One-block W prefetch: the next block's W DMAs are emitted before the
        # current block's matmuls, and the first o-block's W is already in
        # flight before the LayerNorm work.
        allb = blocks + o_blocks

        # ---- load combined^T hi/lo; the m=0 slab goes first, then the first
        # W slice, then the remaining slabs, so the serial DMA startup chain
        # covers exactly what the first matmuls need ----
        def load_ct(m):
            nc.sync.dma_start(out=combH[:, :, m, :],
                              in_=cth_k[:, :, m * 128:(m + 1) * 128])
            nc.sync.dma_start(out=combL[:, :, m, :],
                              in_=ctl_k[:, :, m * 128:(m + 1) * 128])

        load_ct(0)
        w_next = load_w(*allb[0])
        for m in range(1, MCH):
            load_ct(m)

        ln_done = False
        for idx, (gate, q) in enumerate(allb):
            if gate == "o" and not ln_done:
                emit_ln()
                ln_done = True
            w_cur = w_next
            if idx + 1 < len(allb):
                w_next = load_w(*allb[idx + 1])
            do_block(gate, q, *w_cur)

    nc.finalize()
    return nc


def _get_program(trivial_gb: bool):
    if trivial_gb not in _PROGRAMS:
        _PROGRAMS[trivial_gb] = _build_program(trivial_gb)
    return _PROGRAMS[trivial_gb]


def _split2_e4m3(a):
    """2-term fp8 split at a shared scale: a ~ hi + lo (lo mostly subnormal)."""
    hi = a.astype(E4NP)
    lo = (a - hi.astype(np.float32)).astype(E4NP)
    return hi, lo


def kernel(x, h_prev, c_prev, forget_mask, W, b, ln_gamma, ln_beta):
    from concourse.bass_utils import run_bass_kernel_spmd

    f32 = np.float32
    x = np.ascontiguousarray(x, dtype=f32)
    h_prev = np.ascontiguousarray(h_prev, dtype=f32)
    c_prev = np.ascontiguousarray(c_prev, dtype=f32)
    forget_mask = np.ascontiguousarray(forget_mask, dtype=f32)
    W = np.ascontiguousarray(W, dtype=f32)
    b = np.ascontiguousarray(b, dtype=f32)
    ln_gamma = np.ascontiguousarray(ln_gamma, dtype=f32)
    ln_beta = np.ascontiguousarray(ln_beta, dtype=f32)

    trivial_gb = bool(np.all(ln_gamma == 1.0) and np.all(ln_beta == 0.0))
    nc = _get_program(trivial_gb)

    # pre-transposed [x | h_prev] per shard, split to fp8 hi/lo host-side
    comb_t = np.ascontiguousarray(
        np.concatenate((x, h_prev), axis=1).T)  # [IN+H, B]
    ct_hi, ct_lo = _split2_e4m3(comb_t)
    w_hi, w_lo = _split2_e4m3(W * WSCALE)
    cp16 = c_prev.astype(BF16)

    ones_dr = np.zeros((1, 2, 128), dtype=f32)
    ones_dr[0, 0, :] = 4.0
    b_dr = np.zeros((1, 2, 4 * H), dtype=f32)
    b_dr[0, 0, :] = 16.0 * b

    in_maps = []
    for i in range(NCORES):
        sl = slice(i * BC, (i + 1) * BC)
        in_maps.append({
            "combT_hi": np.ascontiguousarray(ct_hi[:, sl]),
            "combT_lo": np.ascontiguousarray(ct_lo[:, sl]),
            "c_prev": cp16[sl],
            "forget_mask": forget_mask[sl].reshape(MCH, 128),
            "W_hi": w_hi,
            "W_lo": w_lo,
            "ones_dr": ones_dr.astype(E4NP),
            "b_dr": b_dr.astype(E4NP),
            "ln_gamma": ln_gamma.reshape(1, H),
            "ln_beta": ln_beta.reshape(1, H),
        })

    res = run_bass_kernel_spmd(nc, in_maps, list(range(NCORES)))
    h_t = np.concatenate(
        [r["h_out"].astype(f32) for r in res.results], axis=0)
    c_t = np.concatenate([r["c_out"] for r in res.results], axis=0)
    return (h_t, c_t)


# revision 9
# speedup vs baseline: 1.3554x; 1.0810x over previous
"""EnhancedLSTMCell Trainium2 kernel.

Data-parallel over 8 NeuronCores: batch B=8192 split into 8 shards of 1024
rows. Per core:
    gates = [x | h_prev] @ W + b          # [1024, 4096]
    i,f,g,o = split(gates); f *= mask
    c = f*c_prev + i*g; c = LayerNorm(c)*gamma + beta; h = o*tanh(c)

The GEMM runs on the PE in fp8e4 (e4m3) DoubleRow perf mode (2 k-tiles per
instruction at 0.5 cycles/row — 4x the fp32r MAC rate). fp32 precision is
recovered with a 2-term split of BOTH operands: v ~ v_hi + v_lo with v_hi =
q8(v) and v_lo = q8(v - v_hi) held at the SAME scale (the residual lands in
e4m3's subnormal range, giving a uniform ~2^-10 absolute floor). Per k-pair
the three significant cross products hi@hi, hi@lo, lo@hi are accumulated
(lo@lo is ~1e-6 relative and dropped), so the GEMM costs 1.5 DR instructions
per k-tile = 0.75x the fp32r cycle count with ~1e-3 relative gate error.
W is pre-scaled by 64 host-side so its fp8 range is normal; the ACT drains
apply scale=1/64. Bias enters each PSUM group via a K=1 DoubleRow matmul of
(ones*4) @ q8(16*b) (= 64*b, rescaled by the same drain). All quantization
happens host-side, so fp8 tiles are DMA'd straight into the PE operands —
no on-device rounding copies at all.

DMA shape discipline: descriptor runs under 512B pay a 2x bus latency
multiplier, and all transfers serialize on the shared DMA engines, so W is
fetched in 512-column double-blocks ([128, 8, 512] fp8 k-half slices),
comb^T hi/lo in three >=256-column slabs each, and c_prev as one whole
[128, 8, 1024] bf16 transfer up front. The W column loop therefore covers
512 cols per step (two 256-wide PSUM groups per m), ordered i01, g01, i23,
g23, f01, f23, LN, o01, o23 with a one-double-block prefetch.

Layout: batch rows on partitions (8 chunks of 128), contraction dim K=2048
on partitions for matmul operands (host feeds [x | h_prev]^T per shard).
c accumulates in SBUF fp32: the i-drain writes sigmoid(i) in place, the
g-drain multiplies tanh(g) in, the f-drain adds (sigmoid(f)*mask)*c_prev.
LayerNorm uses bn_stats/bn_aggr + Sqrt + DVE reciprocal; tanh(c_t)
overwrites the accumulator to feed h = sigmoid(o) * tanh(c_t), written out
as bf16 (c_out stays fp32).

Built on bacc.Bacc (not bass.Bass): Bacc's finalize() legalizes multi-sem
waits that the walrus DMA/LDW instruction encodings cannot carry.
"""

import sys

if "/opt/trn_rl_repo" not in sys.path:
    sys.path.insert(0, "/opt/trn_rl_repo")

import numpy as np
import ml_dtypes

B = 8192
IN = 1024
H = 1024
NCORES = 8
BC = B // NCORES          # 1024 rows per core
MCH = BC // 128           # 8 partition chunks of batch rows
KCH = (IN + H) // 128     # 16 contraction chunks
KP = KCH // 2             # 8 DoubleRow k-pairs
CB = 256                  # PSUM group column width
DB = 512                  # W fetch double-block width (512B fp8 lines)
EPS = 1e-5
WSCALE = 64.0             # W pre-scale; drains divide by 64

E4NP = ml_dtypes.float8_e4m3
BF16 = ml_dtypes.bfloat16

_PROGRAMS = {}


def _build_program(trivial_gb: bool):
    from contextlib import ExitStack

    import concourse.bass as bass
    import concourse.tile as tile
    from concourse import bacc, mybir

    F32 = mybir.dt.float32
    F8 = mybir.dt.float8e4
    BF = mybir.dt.bfloat16
    AF = mybir.ActivationFunctionType
    ALU = mybir.AluOpType
    DR = mybir.MatmulPerfMode.DoubleRow

    nc = bacc.Bacc("TRN2", target_bir_lowering=False, debug=False)

    # combined^T = [x | h_prev]^T per shard, hi/lo fp8 split, transposed
    # host-side so the contraction dim lands on partitions with unit-stride DMA
    cth_d = nc.dram_tensor("combT_hi", [IN + H, BC], F8, kind="ExternalInput").ap()
    ctl_d = nc.dram_tensor("combT_lo", [IN + H, BC], F8, kind="ExternalInput").ap()
    c_d = nc.dram_tensor("c_prev", [BC, H], BF, kind="ExternalInput").ap()
    m_d = nc.dram_tensor("forget_mask", [MCH, 128], F32, kind="ExternalInput").ap()
    wh_d = nc.dram_tensor("W_hi", [IN + H, 4 * H], F8, kind="ExternalInput").ap()
    wl_d = nc.dram_tensor("W_lo", [IN + H, 4 * H], F8, kind="ExternalInput").ap()
    ones_d = nc.dram_tensor("ones_dr", [1, 2, 128], F8, kind="ExternalInput").ap()
    b_d = nc.dram_tensor("b_dr", [1, 2, 4 * H], F8, kind="ExternalInput").ap()
    g_d = nc.dram_tensor("ln_gamma", [1, H], F32, kind="ExternalInput").ap()
    be_d = nc.dram_tensor("ln_beta", [1, H], F32, kind="ExternalInput").ap()
    ho_d = nc.dram_tensor("h_out", [BC, H], BF, kind="ExternalOutput").ap()
    co_d = nc.dram_tensor("c_out", [BC, H], F32, kind="ExternalOutput").ap()

    wh_k = wh_d.rearrange("(k p) n -> p k n", p=128)  # [128, 16, 4096]
    wl_k = wl_d.rearrange("(k p) n -> p k n", p=128)
    cth_k = cth_d.rearrange("(k p) b -> p k b", p=128)  # [128, 16, 1024]
    ctl_k = ctl_d.rearrange("(k p) b -> p k b", p=128)
    cp_k = c_d.rearrange("(m p) h -> p m h", p=128)     # [128, 8, 1024]

    with tile.TileContext(nc) as tc, ExitStack() as ctx:
        singles = ctx.enter_context(tc.tile_pool(name="singles", bufs=1))
        bigs = ctx.enter_context(tc.tile_pool(name="bigs", bufs=1))
        wpool = ctx.enter_context(tc.tile_pool(name="w", bufs=2))
        tpool = ctx.enter_context(tc.tile_pool(name="tmp", bufs=4))
        hpool = ctx.enter_context(tc.tile_pool(name="hout", bufs=8))
        zpool = ctx.enter_context(
            tc.tile_pool(name="z", bufs=4 if trivial_gb else 2))
        pmain = ctx.enter_context(tc.tile_pool(name="pmain", bufs=8, space="PSUM"))

        ones_t = singles.tile([1, 2, 128], F8)
        b_t = singles.tile([1, 2, 4 * H], F8)
        nc.sync.dma_start(out=ones_t, in_=ones_d)
        nc.sync.dma_start(out=b_t, in_=b_d)
        mask_sb = singles.tile([128, MCH], F32)
        nc.sync.dma_start(out=mask_sb, in_=m_d.rearrange("m p -> p m"))
        if not trivial_gb:
            gam_bc = singles.tile([128, H], F32)
            nc.sync.dma_start(
                out=gam_bc,
                in_=bass.AP(tensor=g_d.tensor, offset=g_d.offset,
                            ap=[[0, 128], g_d.ap[1]]),
            )
            bet_bc = singles.tile([128, H], F32)
            nc.sync.dma_start(
                out=bet_bc,
                in_=bass.AP(tensor=be_d.tensor, offset=be_d.offset,
                            ap=[[0, 128], be_d.ap[1]]),
            )

        # combT[k, b] = 128x128 transposed blocks of [x | h_prev], fp8 hi/lo
        combH = bigs.tile([128, KCH, BC], F8)
        combL = bigs.tile([128, KCH, BC], F8)
        cp_all = bigs.tile([128, MCH, H], BF)
        c_acc = bigs.tile([128, MCH, H], F32)
        mvall = singles.tile([128, MCH, 2], F32)
        std_t = singles.tile([128, MCH], F32)
        inv_t = singles.tile([128, MCH], F32)
        eps_t = singles.tile([128, 1], F32)
        nc.vector.memset(eps_t, EPS)

        # ---- W double-block schedule: 512 cols per step ----
        # both i blocks first: the PE then has 43us of work gated only on
        # comb + W_i01 (~17.5us of bus), so the DMA stream runs ahead of the
        # PE for the whole kernel instead of racing it block by block
        GOFF = {"i": 0, "f": H, "g": 2 * H, "o": 3 * H}
        allb = [("i", 0), ("i", 1), ("g", 0), ("g", 1),
                ("f", 0), ("f", 1), ("o", 0), ("o", 1)]

        def load_w(gate, d, nchunk=2):
            # one 512-col double-block, hi/lo, k-sliced (quarters for the
            # first block so its first matmuls start ~1.5us earlier)
            col0 = GOFF[gate] + d * DB
            wh = wpool.tile([128, KCH, DB], F8, tag="wh")
            wl = wpool.tile([128, KCH, DB], F8, tag="wl")
            hk = KCH // nchunk
            for hchunk in range(nchunk):
                ks = slice(hchunk * hk, (hchunk + 1) * hk)
                nc.sync.dma_start(out=wh[:, ks, :],
                                  in_=wh_k[:, ks, col0:col0 + DB])
                nc.sync.dma_start(out=wl[:, ks, :],
                                  in_=wl_k[:, ks, col0:col0 + DB])
            return wh, wl

        def do_block(gate, d, wh, wl):
            inv = 1.0 / WSCALE
            for m in range(MCH):
                for half in range(2):
                    q = 2 * d + half
                    col0 = GOFF[gate] + q * CB
                    c0 = half * CB
                    ps = pmain.tile([128, CB], F32, tag="ps")
                    # bias via K=1 DoubleRow: (ones*4) @ q8(16*b) = 64*b
                    nc.tensor.matmul(ps, ones_t, b_t[:, :, col0:col0 + CB],
                                     start=True, stop=False, perf_mode=DR)
                    for j in range(KP):
                        ks = slice(2 * j, 2 * j + 2)
                        ah = combH[:, ks, m * 128:(m + 1) * 128]
                        al = combL[:, ks, m * 128:(m + 1) * 128]
                        whj = wh[:, ks, c0:c0 + CB]
                        wlj = wl[:, ks, c0:c0 + CB]
                        nc.tensor.matmul(ps, ah, whj, start=False, stop=False,
                                         perf_mode=DR)
                        nc.tensor.matmul(ps, ah, wlj, start=False, stop=False,
                                         perf_mode=DR)
                        nc.tensor.matmul(ps, al, whj, start=False,
                                         stop=(j == KP - 1), perf_mode=DR)
                    csl = c_acc[:, m, q * CB:(q + 1) * CB]
                    if gate == "i":
                        nc.scalar.activation(csl, ps, AF.Sigmoid, scale=inv)
                    elif gate == "g":
                        tg = tpool.tile([128, CB], F32, tag="t")
                        nc.scalar.activation(tg, ps, AF.Tanh, scale=inv)
                        nc.vector.tensor_mul(csl, csl, tg)
                    elif gate == "f":
                        tf = tpool.tile([128, CB], F32, tag="t")
                        nc.scalar.activation(tf, ps, AF.Sigmoid, scale=inv)
                        t2 = tpool.tile([128, CB], F32, tag="t")
                        nc.vector.scalar_tensor_tensor(
                            t2, tf, mask_sb[:, m:m + 1],
                            cp_all[:, m, q * CB:(q + 1) * CB],
                            ALU.mult, ALU.mult)
                        nc.vector.tensor_add(csl, csl, t2)
                    else:  # o
                        to = tpool.tile([128, CB], F32, tag="t")
                        nc.scalar.activation(to, ps, AF.Sigmoid, scale=inv)
                        hh = hpool.tile([128, CB], BF, tag="h")
                        nc.vector.tensor_mul(hh, to, csl)  # csl = tanh(c_t)
                        nc.sync.dma_start(
                            out=ho_d[m * 128:(m + 1) * 128,
                                     q * CB:(q + 1) * CB],
                            in_=hh)

        def emit_ln():
            # ---- LayerNorm over H per m-chunk ----
            for m in range(MCH):
                st = tpool.tile([128, 2, 6], F32, tag="st")
                for hf in range(2):
                    nc.vector.bn_stats(
                        out=st[:, hf, :],
                        in_=c_acc[:, m, hf * 512:(hf + 1) * 512])
                nc.vector.bn_aggr(out=mvall[:, m, :], in_=st)
            # std = sqrt(var + eps); inv = 1/std
            nc.scalar.activation(std_t, mvall[:, :, 1], AF.Sqrt, bias=eps_t)
            nc.vector.reciprocal(inv_t, std_t)
            for m in range(MCH):
                z = zpool.tile([128, H], F32, tag="z")
                nc.vector.tensor_scalar(
                    z, c_acc[:, m, :], mvall[:, m, 0:1], inv_t[:, m:m + 1],
                    ALU.subtract, ALU.mult)
                if not trivial_gb:
                    nc.vector.tensor_mul(z, z, gam_bc)
                    nc.vector.tensor_add(z, z, bet_bc)
                nc.sync.dma_start(out=co_d[m * 128:(m + 1) * 128, :], in_=z)
                nc.scalar.activation(c_acc[:, m, :], z, AF.Tanh)

        # ---- startup DMA order: feed the PE's first groups first ----
        # comb slabs stay >=512 cols wide (descriptor runs under 512B pay a
        # 2x bus penalty): m0-3 first (k-halved so the first products start
        # early), then m4-7; c_prev is queued only after the g01 W prefetch
        # (not needed till the f blocks).
        def load_ct(b0, b1, k0, k1):
            nc.sync.dma_start(out=combH[:, k0:k1, b0:b1],
                              in_=cth_k[:, k0:k1, b0:b1])
            nc.sync.dma_start(out=combL[:, k0:k1, b0:b1],
                              in_=ctl_k[:, k0:k1, b0:b1])

        load_ct(0, 512, 0, KCH // 2)
        load_ct(0, 512, KCH // 2, KCH)
        w_next = load_w(*allb[0], nchunk=4)
        load_ct(512, 1024, 0, KCH)

        ln_done = False
        for idx, (gate, d) in enumerate(allb):
            if gate == "o" and not ln_done:
                emit_ln()
                ln_done = True
            w_cur = w_next
            if idx + 1 < len(allb):
                w_next = load_w(*allb[idx + 1])
            if idx == 3:
                nc.sync.dma_start(out=cp_all, in_=cp_k)
            do_block(gate, d, *w_cur)

    nc.finalize()
    return nc


def _get_program(trivial_gb: bool):
    if trivial_gb not in _PROGRAMS:
        _PROGRAMS[trivial_gb] = _build_program(trivial_gb)
    return _PROGRAMS[trivial_gb]


def _split2_e4m3(a):
    """2-term fp8 split at a shared scale: a ~ hi + lo (lo mostly subnormal)."""
    hi = a.astype(E4NP)
    lo = (a - hi.astype(np.float32)).astype(E4NP)
    return hi, lo


def kernel(x, h_prev, c_prev, forget_mask, W, b, ln_gamma, ln_beta):
    from concourse.bass_utils import run_bass_kernel_spmd

    f32 = np.float32
    x = np.ascontiguousarray(x, dtype=f32)
    h_prev = np.ascontiguousarray(h_prev, dtype=f32)
    c_prev = np.ascontiguousarray(c_prev, dtype=f32)
    forget_mask = np.ascontiguousarray(forget_mask, dtype=f32)
    W = np.ascontiguousarray(W, dtype=f32)
    b = np.ascontiguousarray(b, dtype=f32)
    ln_gamma = np.ascontiguousarray(ln_gamma, dtype=f32)
    ln_beta = np.ascontiguousarray(ln_beta, dtype=f32)

    trivial_gb = bool(np.all(ln_gamma == 1.0) and np.all(ln_beta == 0.0))
    nc = _get_program(trivial_gb)

    # pre-transposed [x | h_prev] per shard, split to fp8 hi/lo host-side
    comb_t = np.ascontiguousarray(
        np.concatenate((x, h_prev), axis=1).T)  # [IN+H, B]
    ct_hi, ct_lo = _split2_e4m3(comb_t)
    w_hi, w_lo = _split2_e4m3(W * WSCALE)
    cp16 = c_prev.astype(BF16)

    ones_dr = np.zeros((1, 2, 128), dtype=f32)
    ones_dr[0, 0, :] = 4.0
    b_dr = np.zeros((1, 2, 4 * H), dtype=f32)
    b_dr[0, 0, :] = 16.0 * b

    in_maps = []
    for i in range(NCORES):
        sl = slice(i * BC, (i + 1) * BC)
        in_maps.append({
            "combT_hi": np.ascontiguousarray(ct_hi[:, sl]),
            "combT_lo": np.ascontiguousarray(ct_lo[:, sl]),
            "c_prev": cp16[sl],
            "forget_mask": forget_mask[sl].reshape(MCH, 128),
            "W_hi": w_hi,
            "W_lo": w_lo,
            "ones_dr": ones_dr.astype(E4NP),
            "b_dr": b_dr.astype(E4NP),
            "ln_gamma": ln_gamma.reshape(1, H),
            "ln_beta": ln_beta.reshape(1, H),
        })

    res = run_bass_kernel_spmd(nc, in_maps, list(range(NCORES)))
    h_t = np.concatenate(
        [r["h_out"].astype(f32) for r in res.results], axis=0)
    c_t = np.concatenate([r["c_out"] for r in res.results], axis=0)
    return (h_t, c_t)


# revision 32
# speedup vs baseline: 1.4116x; 1.0414x over previous
"""EnhancedLSTMCell Trainium2 kernel.

Data-parallel over 8 NeuronCores: batch B=8192 split into 8 shards of 1024
rows. Per core:
    gates = [x | h_prev] @ W + b          # [1024, 4096]
    i,f,g,o = split(gates); f *= mask
    c = f*c_prev + i*g; c = LayerNorm(c)*gamma + beta; h = o*tanh(c)

The GEMM runs on the PE in fp8e4 (e4m3) DoubleRow perf mode (2 k-tiles per
instruction at 0.5 cycles/row — 4x the fp32r MAC rate). fp32 precision is
recovered with a 2-term split of BOTH operands: v ~ v_hi + v_lo with v_hi =
q8(v) and v_lo = q8(v - v_hi) held at the SAME scale (the residual lands in
e4m3's subnormal range, giving a uniform ~2^-10 absolute floor). Per k-pair
the three significant cross products hi@hi, hi@lo, lo@hi are accumulated
(lo@lo is ~1e-6 relative and dropped), so the GEMM costs 1.5 DR instructions
per k-tile = 0.75x the fp32r cycle count with ~1e-3 relative gate error.
W is pre-scaled by 64 host-side so its fp8 range is normal; the ACT drains
apply scale=1/64. Bias enters each PSUM group via a K=1 DoubleRow matmul of
(ones*4) @ q8(16*b) (= 64*b, rescaled by the same drain). All quantization
happens host-side, so fp8 tiles are DMA'd straight into the PE operands —
no on-device rounding copies at all.

DMA shape discipline: descriptor runs under 512B pay a 2x bus latency
multiplier, and all transfers serialize on the shared DMA engines, so W is
fetched in 512-column double-blocks ([128, 8, 512] fp8 k-half slices),
comb^T hi/lo in three >=256-column slabs each, and c_prev as one whole
[128, 8, 1024] bf16 transfer up front. The W column loop therefore covers
512 cols per step (two 256-wide PSUM groups per m), ordered i01, g01, i23,
g23, f01, f23, LN, o01, o23 with a one-double-block prefetch.

Layout: batch rows on partitions (8 chunks of 128), contraction dim K=2048
on partitions for matmul operands (host feeds [x | h_prev]^T per shard).
c accumulates in SBUF fp32: the i-drain writes sigmoid(i) in place, the
g-drain multiplies tanh(g) in, the f-drain adds (sigmoid(f)*mask)*c_prev.
LayerNorm uses bn_stats/bn_aggr + Sqrt + DVE reciprocal; tanh(c_t)
overwrites the accumulator to feed h = sigmoid(o) * tanh(c_t), written out
as bf16 (c_out stays fp32).

Built on bacc.Bacc (not bass.Bass): Bacc's finalize() legalizes multi-sem
waits that the walrus DMA/LDW instruction encodings cannot carry.
"""

import sys

if "/opt/trn_rl_repo" not in sys.path:
    sys.path.insert(0, "/opt/trn_rl_repo")

import numpy as np
import ml_dtypes

B = 8192
IN = 1024
H = 1024
NCORES = 8
BC = B // NCORES          # 1024 rows per core
MCH = BC // 128           # 8 partition chunks of batch rows
KCH = (IN + H) // 128     # 16 contraction chunks
KP = KCH // 2             # 8 DoubleRow k-pairs
CB = 256                  # PSUM group column width
DB = 512                  # W fetch double-block width (512B fp8 lines)
EPS = 1e-5
WSCALE = 64.0             # W pre-scale; drains divide by 64

E4NP = ml_dtypes.float8_e4m3
BF16 = ml_dtypes.bfloat16

_PROGRAMS = {}


def _build_program(trivial_gb: bool):
    from contextlib import ExitStack

    import concourse.bass as bass
    import concourse.tile as tile
    from concourse import bacc, mybir

    F32 = mybir.dt.float32
    F32R = mybir.dt.float32r
    F8 = mybir.dt.float8e4
    BF = mybir.dt.bfloat16
    AF = mybir.ActivationFunctionType
    ALU = mybir.AluOpType
    DR = mybir.MatmulPerfMode.DoubleRow

    nc = bacc.Bacc("TRN2", target_bir_lowering=False, debug=False)

    # combined^T = [x | h_prev]^T per shard, hi/lo fp8 split, transposed
    # host-side so the contraction dim lands on partitions with unit-stride DMA
    cth_d = nc.dram_tensor("combT_hi", [IN + H, BC], F8, kind="ExternalInput").ap()
    ctl_d = nc.dram_tensor("combT_lo", [IN + H, BC], F8, kind="ExternalInput").ap()
    c_d = nc.dram_tensor("c_prev", [BC, H], BF, kind="ExternalInput").ap()
    m_d = nc.dram_tensor("forget_mask", [MCH, 128], F32, kind="ExternalInput").ap()
    wh_d = nc.dram_tensor("W_hi", [IN + H, 4 * H], F8, kind="ExternalInput").ap()
    wl_d = nc.dram_tensor("W_lo", [IN + H, 4 * H], F8, kind="ExternalInput").ap()
    b_d = nc.dram_tensor("b64", [1, 4 * H], F32, kind="ExternalInput").ap()
    ones_d = nc.dram_tensor("ones_dr", [1, 2, 128], F8, kind="ExternalInput").ap()
    bq_d = nc.dram_tensor("b_dr", [1, 2, 4 * H], F8, kind="ExternalInput").ap()
    g_d = nc.dram_tensor("ln_gamma", [1, H], F32, kind="ExternalInput").ap()
    be_d = nc.dram_tensor("ln_beta", [1, H], F32, kind="ExternalInput").ap()
    ho_d = nc.dram_tensor("h_out", [BC, H], BF, kind="ExternalOutput").ap()
    co_d = nc.dram_tensor("c_out", [BC, H], F32, kind="ExternalOutput").ap()

    wh_k = wh_d.rearrange("(k p) n -> p k n", p=128)  # [128, 16, 4096]
    wl_k = wl_d.rearrange("(k p) n -> p k n", p=128)
    cth_k = cth_d.rearrange("(k p) b -> p k b", p=128)  # [128, 16, 1024]
    ctl_k = ctl_d.rearrange("(k p) b -> p k b", p=128)
    cp_k = c_d.rearrange("(m p) h -> p m h", p=128)     # [128, 8, 1024]

    with tile.TileContext(nc) as tc, ExitStack() as ctx:
        singles = ctx.enter_context(tc.tile_pool(name="singles", bufs=1))
        bigs = ctx.enter_context(tc.tile_pool(name="bigs", bufs=1))
        wpool = ctx.enter_context(tc.tile_pool(name="w", bufs=2))
        tpool = ctx.enter_context(tc.tile_pool(name="tmp", bufs=4))
        hpool = ctx.enter_context(tc.tile_pool(name="hout", bufs=8))
        zpool = ctx.enter_context(
            tc.tile_pool(name="z", bufs=4 if trivial_gb else 2))
        pmain = ctx.enter_context(tc.tile_pool(name="pmain", bufs=8, space="PSUM"))

        # bias 64*b broadcast to all partitions once: a K=1 fp32r ones-matmul
        # into PSUM, drained to SBUF by ACT — runs in the idle pre-comb window
        b_full = singles.tile([128, 4 * H], F32)
        b_sb = singles.tile([1, 4 * H], F32)
        nc.sync.dma_start(out=b_sb, in_=b_d)
        ones_f = singles.tile([1, 128], F32)
        nc.vector.memset(ones_f, 1.0)
        ones_r = singles.tile([1, 128], F32R)
        nc.scalar.copy(ones_r, ones_f)
        b_r = singles.tile([1, 4 * H], F32R)

        def bcast_bias(cc):
            # one 256-col chunk of the b64 partition-broadcast: fp32r
            # rounding copy (BIR requires a rounding producer), K=1 fp32r
            # ones-matmul, ACT drain to b_full. Emitted just-in-time per
            # block so the PE queue never waits on the ACT-paced copies.
            cs = slice(cc * CB, (cc + 1) * CB)
            nc.scalar.copy(b_r[:, cs], b_sb[:, cs])
            ps_b = pmain.tile([128, CB], F32, tag="ps")
            nc.tensor.matmul(ps_b, ones_r, b_r[:, cs], start=True, stop=True)
            nc.scalar.copy(b_full[:, cs], ps_b)
        ones_t = singles.tile([1, 2, 128], F8)
        b_t = singles.tile([1, 2, 4 * H], F8)
        mask_sb = singles.tile([128, MCH], F32)
        if not trivial_gb:
            gam_bc = singles.tile([128, H], F32)
            nc.sync.dma_start(
                out=gam_bc,
                in_=bass.AP(tensor=g_d.tensor, offset=g_d.offset,
                            ap=[[0, 128], g_d.ap[1]]),
            )
            bet_bc = singles.tile([128, H], F32)
            nc.sync.dma_start(
                out=bet_bc,
                in_=bass.AP(tensor=be_d.tensor, offset=be_d.offset,
                            ap=[[0, 128], be_d.ap[1]]),
            )

        # combT[k, b] = 128x128 transposed blocks of [x | h_prev], fp8 hi/lo
        combH = bigs.tile([128, KCH, BC], F8)
        combL = bigs.tile([128, KCH, BC], F8)
        cp_all = bigs.tile([128, MCH, H], BF)
        c_acc = bigs.tile([128, MCH, H], F32)
        mvall = singles.tile([128, MCH, 2], F32)
        std_t = singles.tile([128, MCH], F32)
        inv_t = singles.tile([128, MCH], F32)
        eps_t = singles.tile([128, 1], F32)
        nc.vector.memset(eps_t, EPS)

        # ---- W double-block schedule: 512 cols per step ----
        # both i blocks first: the PE then has 43us of work gated only on
        # comb + W_i01 (~17.5us of bus), so the DMA stream runs ahead of the
        # PE for the whole kernel instead of racing it block by block
        GOFF = {"i": 0, "f": H, "g": 2 * H, "o": 3 * H}
        allb = [("i", 0), ("i", 1), ("g", 0), ("g", 1),
                ("f", 0), ("f", 1), ("o", 0), ("o", 1)]

        def load_w(gate, d, nchunk=2):
            # one 512-col double-block, hi/lo, k-sliced (quarters for the
            # first block so its first matmuls start ~1.5us earlier)
            col0 = GOFF[gate] + d * DB
            wh = wpool.tile([128, KCH, DB], F8, tag="wh")
            wl = wpool.tile([128, KCH, DB], F8, tag="wl")
            hk = KCH // nchunk
            for hchunk in range(nchunk):
                ks = slice(hchunk * hk, (hchunk + 1) * hk)
                nc.sync.dma_start(out=wh[:, ks, :],
                                  in_=wh_k[:, ks, col0:col0 + DB])
                nc.sync.dma_start(out=wl[:, ks, :],
                                  in_=wl_k[:, ks, col0:col0 + DB])
            return wh, wl

        def do_block(gate, d, wh, wl):
            inv = 1.0 / WSCALE
            for m in range(MCH):
                for half in range(2):
                    q = 2 * d + half
                    col0 = GOFF[gate] + q * CB
                    c0 = half * CB
                    ps = pmain.tile([128, CB], F32, tag="ps")
                    # bias pre-written into PSUM by DVE (idle during i/g/f
                    # accumulation), products accumulate on top (start=False);
                    # o keeps the K=1 DoubleRow ones-matmul bias because its
                    # DVE copies would queue behind the LayerNorm chain
                    skip = gate != "o"
                    if gate == "o":
                        nc.tensor.matmul(ps, ones_t, b_t[:, :, col0:col0 + CB],
                                         start=True, stop=False, perf_mode=DR)
                    else:
                        nc.vector.tensor_copy(ps, b_full[:, col0:col0 + CB])
                    for j in range(KP):
                        ks = slice(2 * j, 2 * j + 2)
                        ah = combH[:, ks, m * 128:(m + 1) * 128]
                        al = combL[:, ks, m * 128:(m + 1) * 128]
                        whj = wh[:, ks, c0:c0 + CB]
                        wlj = wl[:, ks, c0:c0 + CB]
                        nc.tensor.matmul(ps, ah, whj, start=False, stop=False,
                                         perf_mode=DR, skip_group_check=skip)
                        nc.tensor.matmul(ps, ah, wlj, start=False, stop=False,
                                         perf_mode=DR, skip_group_check=skip)
                        nc.tensor.matmul(ps, al, whj, start=False,
                                         stop=(j == KP - 1), perf_mode=DR,
                                         skip_group_check=skip)
                    csl = c_acc[:, m, q * CB:(q + 1) * CB]
                    if gate == "i":
                        nc.scalar.activation(csl, ps, AF.Sigmoid, scale=inv)
                    elif gate == "g":
                        tg = tpool.tile([128, CB], F32, tag="t")
                        nc.scalar.activation(tg, ps, AF.Tanh, scale=inv)
                        nc.vector.tensor_mul(csl, csl, tg)
                    elif gate == "f":
                        tf = tpool.tile([128, CB], F32, tag="t")
                        nc.scalar.activation(tf, ps, AF.Sigmoid, scale=inv)
                        t2 = tpool.tile([128, CB], F32, tag="t")
                        nc.vector.scalar_tensor_tensor(
                            t2, tf, mask_sb[:, m:m + 1],
                            cp_all[:, m, q * CB:(q + 1) * CB],
                            ALU.mult, ALU.mult)
                        nc.vector.tensor_add(csl, csl, t2)
                    else:  # o
                        to = tpool.tile([128, CB], F32, tag="t")
                        nc.scalar.activation(to, ps, AF.Sigmoid, scale=inv)
                        hh = hpool.tile([128, CB], BF, tag="h")
                        nc.vector.tensor_mul(hh, to, csl)  # csl = tanh(c_t)
                        nc.sync.dma_start(
                            out=ho_d[m * 128:(m + 1) * 128,
                                     q * CB:(q + 1) * CB],
                            in_=hh)

        def emit_ln():
            # ---- LayerNorm over H per m-chunk ----
            for m in range(MCH):
                st = tpool.tile([128, 2, 6], F32, tag="st")
                for hf in range(2):
                    nc.vector.bn_stats(
                        out=st[:, hf, :],
                        in_=c_acc[:, m, hf * 512:(hf + 1) * 512])
                nc.vector.bn_aggr(out=mvall[:, m, :], in_=st)
            # std = sqrt(var + eps); inv = 1/std
            nc.scalar.activation(std_t, mvall[:, :, 1], AF.Sqrt, bias=eps_t)
            nc.vector.reciprocal(inv_t, std_t)
            for m in range(MCH):
                z = zpool.tile([128, H], F32, tag="z")
                nc.vector.tensor_scalar(
                    z, c_acc[:, m, :], mvall[:, m, 0:1], inv_t[:, m:m + 1],
                    ALU.subtract, ALU.mult)
                if not trivial_gb:
                    nc.vector.tensor_mul(z, z, gam_bc)
                    nc.vector.tensor_add(z, z, bet_bc)
                nc.sync.dma_start(out=co_d[m * 128:(m + 1) * 128, :], in_=z)
                nc.scalar.activation(c_acc[:, m, :], z, AF.Tanh)

        # ---- startup DMA order: feed the PE's first groups first ----
        # comb slabs stay >=512 cols wide (descriptor runs under 512B pay a
        # 2x bus penalty): m0-3 first (k-halved so the first products start
        # early), then m4-7; c_prev is queued only after the g01 W prefetch
        # (not needed till the f blocks).
        def load_ct(b0, b1, k0, k1):
            nc.sync.dma_start(out=combH[:, k0:k1, b0:b1],
                              in_=cth_k[:, k0:k1, b0:b1])
            nc.sync.dma_start(out=combL[:, k0:k1, b0:b1],
                              in_=ctl_k[:, k0:k1, b0:b1])

        def load_w1():
            # first double-block interleaved with the comb k-halves by need:
            # products j0-1 need comb-kh1 + W-q1, j2-3 +W-q2, j4-7 +kh2/q3/q4
            gate, d = allb[0]
            col0 = GOFF[gate] + d * DB
            wh = wpool.tile([128, KCH, DB], F8, tag="wh")
            wl = wpool.tile([128, KCH, DB], F8, tag="wl")

            def wq(q):
                ks = slice(q * 4, (q + 1) * 4)
                nc.sync.dma_start(out=wh[:, ks, :],
                                  in_=wh_k[:, ks, col0:col0 + DB])
                nc.sync.dma_start(out=wl[:, ks, :],
                                  in_=wl_k[:, ks, col0:col0 + DB])

            load_ct(0, 512, 0, KCH // 2)
            wq(0)
            wq(1)
            load_ct(0, 512, KCH // 2, KCH)
            wq(2)
            wq(3)
            return wh, wl

        w_next = load_w1()
        load_ct(512, 1024, 0, KCH)
        # small late-use inputs go after the critical startup stream
        nc.sync.dma_start(out=ones_t, in_=ones_d)
        nc.sync.dma_start(out=b_t, in_=bq_d)
        nc.sync.dma_start(out=mask_sb, in_=m_d.rearrange("m p -> p m"))

        def bcast_block(gate, d):
            if gate != "o":
                col0 = GOFF[gate] + d * DB
                bcast_bias(col0 // CB)
                bcast_bias(col0 // CB + 1)

        bcast_block(*allb[0])
        ln_done = False
        for idx, (gate, d) in enumerate(allb):
            if gate == "o" and not ln_done:
                emit_ln()
                ln_done = True
            w_cur = w_next
            if idx + 1 < len(allb):
                w_next = load_w(*allb[idx + 1])
                bcast_block(*allb[idx + 1])
            if idx == 3:
                nc.sync.dma_start(out=cp_all, in_=cp_k)
            do_block(gate, d, *w_cur)

    nc.finalize()
    return nc


def _get_program(trivial_gb: bool):
    if trivial_gb not in _PROGRAMS:
        _PROGRAMS[trivial_gb] = _build_program(trivial_gb)
    return _PROGRAMS[trivial_gb]


def _split2_e4m3(a):
    """2-term fp8 split at a shared scale: a ~ hi + lo (lo mostly subnormal)."""
    hi = a.astype(E4NP)
    lo = (a - hi.astype(np.float32)).astype(E4NP)
    return hi, lo


def kernel(x, h_prev, c_prev, forget_mask, W, b, ln_gamma, ln_beta):
    from concourse.bass_utils import run_bass_kernel_spmd

    f32 = np.float32
    x = np.ascontiguousarray(x, dtype=f32)
    h_prev = np.ascontiguousarray(h_prev, dtype=f32)
    c_prev = np.ascontiguousarray(c_prev, dtype=f32)
    forget_mask = np.ascontiguousarray(forget_mask, dtype=f32)
    W = np.ascontiguousarray(W, dtype=f32)
    b = np.ascontiguousarray(b, dtype=f32)
    ln_gamma = np.ascontiguousarray(ln_gamma, dtype=f32)
    ln_beta = np.ascontiguousarray(ln_beta, dtype=f32)

    trivial_gb = bool(np.all(ln_gamma == 1.0) and np.all(ln_beta == 0.0))
    nc = _get_program(trivial_gb)

    # pre-transposed [x | h_prev] per shard, split to fp8 hi/lo host-side
    comb_t = np.ascontiguousarray(
        np.concatenate((x, h_prev), axis=1).T)  # [IN+H, B]
    ct_hi, ct_lo = _split2_e4m3(comb_t)
    w_hi, w_lo = _split2_e4m3(W * WSCALE)
    cp16 = c_prev.astype(BF16)
    b64 = (WSCALE * b).reshape(1, 4 * H)
    ones_dr = np.zeros((1, 2, 128), dtype=f32)
    ones_dr[0, 0, :] = 4.0
    b_dr = np.zeros((1, 2, 4 * H), dtype=f32)
    b_dr[0, 0, :] = 16.0 * b

    in_maps = []
    for i in range(NCORES):
        sl = slice(i * BC, (i + 1) * BC)
        in_maps.append({
            "combT_hi": np.ascontiguousarray(ct_hi[:, sl]),
            "combT_lo": np.ascontiguousarray(ct_lo[:, sl]),
            "c_prev": cp16[sl],
            "forget_mask": forget_mask[sl].reshape(MCH, 128),
            "W_hi": w_hi,
            "W_lo": w_lo,
            "b64": b64,
            "ones_dr": ones_dr.astype(E4NP),
            "b_dr": b_dr.astype(E4NP),
            "ln_gamma": ln_gamma.reshape(1, H),
            "ln_beta": ln_beta.reshape(1, H),
        })

    res = run_bass_kernel_spmd(nc, in_maps, list(range(NCORES)))
    h_t = np.concatenate(
        [r["h_out"].astype(f32) for r in res.results], axis=0)
    c_t = np.concatenate([r["c_out"] for r in res.results], axis=0)
    return (h_t, c_t)
